# revision 14
# baseline (speedup 1.0000x reference)
"""XNOR-Net conv2d kernel for Trainium2.

Computes conv2d(sign(x), sign(W), stride=1, pad=1) * alpha for
x:(32,256,56,56) f32, W:(256,256,3,3) f32, alpha:(256,1,1) f32.

Strategy: data-parallel over batch (4 images per core x 8 cores).
Per core, implicit GEMM on the PE array in fp8 (DoubleRow, K=256).
sign(x) is +-1 in fp8 (exact); weights are carried as +-0.5 (one-pass
DVE compute (w>0)-0.5); the missing x2 is folded into alpha at
copyback, so results are bit-exact vs the reference.

Hybrid algorithm per image: rows [0, 40) direct (9 taps per 8-row
chunk, N=448 matmuls), rows [40, 56) via 1D Winograd F(2,3) along
width (12 taps per 16-row chunk, N=448), cutting PE cycles 1.5x for
that band. All Winograd values are exact in fp8: z in {-2..2},
transformed weights in {+-0.25,+-0.75} (built on DVE from the
transposed direct taps: j1=(t0+t1+t2)/2, j2=j1-t1), PSUM accumulates
multiples of 1/4 in fp32, so the result stays bit-exact.

Engine budget per image (target < PE ~22us): PE = 2x(5 direct + 1
wino group); DVE = forward transform + inverse adds + half the direct
copybacks; ACT = signs + other half + wino alpha + the m1 PSUM->SBUF
stage (DVE tensor_tensor may read only one PSUM operand). DMA: few
large transfers (12.5KB/partition image lines, 4.6KB weight lines).
Emission order per engine matches expected execution order (queues
are in-order; a blocked head stalls the queue).
"""

import sys

sys.path.insert(0, "/opt/trn_rl_repo")

import numpy as np

import concourse.bass as bass
import concourse.mybir as mybir
from concourse import bacc
from concourse.bass_utils import run_bass_kernel_spmd
from concourse.masks import make_identity
from concourse.tile import TileContext

P = 128
N_CORES = 8
N_IMG = 32
IMG_PER_CORE = N_IMG // N_CORES
C = 256
H = W = 56
HP = 58  # padded rows (0..57)
WS = 64  # row stride of padded buffer (cols 0..57 used, 58+ never read)
CHUNK = 8  # direct-path output rows per matmul tile -> N = 8*56 = 448
NT = W // 2  # winograd tiles per row (28)
FP8 = mybir.dt.float8e4
F32 = mybir.dt.float32
BF16 = mybir.dt.bfloat16

WINO = 16  # rows per image via Winograd F(2,3); 0 or 16
D = H - WINO  # direct rows 0..D-1
ND = D // CHUNK  # direct chunks
ZR = WINO + 2  # z rows needed (output rows D..55 + kh halo)
NTAP = 15 if WINO else 9  # 9 direct taps (+6 winograd j1/j2 taps)
ACCB = 4 if WINO else 8  # direct-acc PSUM ring (also transpose targets)
WACCB = 2  # winograd acc-pair ring (2 banks each)

last_result = None  # stash of BassKernelResults for test harnesses


def build_conv_kernel():
    nc = bacc.Bacc()
    x_in = nc.declare_dram_parameter(
        "x", [IMG_PER_CORE, C, H, W], F32, isOutput=False
    )
    w_in = nc.declare_dram_parameter("w", [C, C, 3, 3], F32, isOutput=False)
    a_in = nc.declare_dram_parameter("alpha", [C, 1, 1], F32, isOutput=False)
    y_out = nc.declare_dram_parameter(
        "y", [IMG_PER_CORE, C, H, W], F32, isOutput=True
    )
    x_ap, w_ap, a_ap, y_ap = x_in[:], w_in[:], a_in[:], y_out[:]

    with TileContext(nc) as tc:
        with (
            tc.tile_pool(name="wpool", bufs=1) as wpool,
            tc.tile_pool(name="xpool", bufs=3) as xpool,
            tc.tile_pool(name="opool", bufs=6) as opool,
            tc.tile_pool(name="pp", bufs=1, space="PSUM") as pp,
        ):
            # warm up the ACT function table while the first DMAs run
            warm = wpool.tile([P, 1], F32, name="warm")
            nc.vector.memset(warm, 0.0)
            nc.scalar.sign(warm, warm)

            ident = wpool.tile([P, P], BF16, name="ident")
            make_identity(nc, ident)
            alpha_sb = wpool.tile([P, 2], F32, name="alpha_sb")
            nc.sync.dma_start(
                out=alpha_sb, in_=a_ap.flatten().rearrange("(mt co) -> co mt", co=P)
            )
            # weights carry +-0.5; restore the factor 2 here
            nc.vector.tensor_scalar(
                out=alpha_sb,
                in0=alpha_sb,
                scalar1=2.0,
                scalar2=None,
                op0=mybir.AluOpType.mult,
            )

            # [ci_lo, cg, mt, tap, co]; taps 0-8 = (kh*3+kw), taps 9-14 =
            # winograd (9+kh*2 = j1, 10+kh*2 = j2)
            w_lhsT = wpool.tile([P, 2, 2, NTAP, P], FP8, name="w_lhsT")

            wsrcs = {}
            wsgns = {}

            def emit_wdma(mt, cg):
                # [co, ci*9] with 4.6KB contiguous lines
                wsrc = wpool.tile([P, P * 9], F32, name="wsrc", bufs=4)
                nc.sync.dma_start(
                    out=wsrc,
                    in_=w_ap[
                        mt * P : (mt + 1) * P, cg * P : (cg + 1) * P
                    ].rearrange("co ci kh kw -> co (ci kh kw)"),
                )
                wsrcs[(mt, cg)] = wsrc

            def emit_wsgn(mt, cg):
                # one-pass half-sign on DVE: (w > 0) - 0.5 -> +-0.5
                wsgn = wpool.tile([P, P * 9], BF16, name="wsgn", bufs=4)
                nc.vector.tensor_scalar(
                    out=wsgn,
                    in0=wsrcs[(mt, cg)],
                    scalar1=0.0,
                    scalar2=0.5,
                    op0=mybir.AluOpType.is_gt,
                    op1=mybir.AluOpType.subtract,
                )
                wsgns[(mt, cg)] = wsgn

            def emit_wtrans(mt):
                # tap-major, cg-interleaved so tap k (both cg planes) is
                # ready early; transpose targets alias direct-acc PSUM
                # slots (bf16 view); copies to fp8 all on DVE
                for tap in range(9):
                    for cg in range(2):
                        wsgn_v = wsgns[(mt, cg)].rearrange(
                            "p (ci t) -> p ci t", t=9
                        )
                        accb = pp.tile([P, CHUNK * W], F32, name="acc", bufs=ACCB)
                        tpv = accb[:, 0:64].bitcast(BF16)
                        nc.tensor.transpose(tpv, wsgn_v[:, :, tap], ident)
                        nc.vector.tensor_copy(
                            out=w_lhsT[:, cg, mt, tap, :], in_=tpv
                        )

            def emit_wino_taps(mt):
                # winograd taps from transposed direct taps, on DVE:
                # j1 = (t0+t1+t2)/2, j2 = j1-t1 (+-0.25/+-0.75; exact)
                for cg in range(2):
                    dir_v = w_lhsT[:, cg, mt, 0:9, :].rearrange(
                        "p (kh kw) co -> p kh kw co", kw=3
                    )
                    win_v = w_lhsT[:, cg, mt, 9:15, :].rearrange(
                        "p (kh j) co -> p kh j co", j=2
                    )
                    wu = wpool.tile([P, 3, P], FP8, name="wu", bufs=2)
                    wv = wpool.tile([P, 3, P], FP8, name="wv", bufs=2)
                    nc.vector.tensor_add(wu, dir_v[:, :, 0, :], dir_v[:, :, 2, :])
                    nc.vector.tensor_add(wv, wu, dir_v[:, :, 1, :])
                    nc.vector.tensor_scalar_mul(
                        out=win_v[:, :, 0, :], in0=wv, scalar1=0.5
                    )
                    nc.vector.tensor_sub(
                        win_v[:, :, 1, :], win_v[:, :, 0, :], dir_v[:, :, 1, :]
                    )

            xpads = {}
            zbufs = {}

            def emit_xpad(img):
                xpad = xpool.tile([P, 2, HP, WS], FP8, name="xpad", bufs=3)
                xpads[img] = xpad
                nc.vector.memset(xpad[:, :, 0, 0:58], 0.0)
                nc.vector.memset(xpad[:, :, HP - 1, 0:58], 0.0)
                nc.vector.memset(xpad[:, :, 1 : HP - 1, 0], 0.0)
                nc.vector.memset(xpad[:, :, 1 : HP - 1, 57], 0.0)

            def emit_load(img, cg):
                xsrc = xpool.tile([P, H * W], F32, name="xsrc", bufs=4)
                nc.sync.dma_start(
                    out=xsrc,
                    in_=x_ap[img, cg * P : (cg + 1) * P].rearrange(
                        "c h w -> c (h w)"
                    ),
                )
                return xsrc

            def emit_load_q(img, cg, q):
                xq = xpool.tile([P, 14 * W], F32, name="xsrcq", bufs=4)
                nc.sync.dma_start(
                    out=xq,
                    in_=x_ap[
                        img, cg * P : (cg + 1) * P, q * 14 : (q + 1) * 14
                    ].rearrange("c h w -> c (h w)"),
                )
                return xq

            def emit_sign(img, cg, r0, nr, src):
                nc.scalar.sign(
                    xpads[img][:, cg, r0 + 1 : r0 + 1 + nr, 1 : W + 1],
                    src.rearrange("p (h w) -> p h w", w=W),
                )

            def emit_fwd(img):
                # forward winograd transform of padded rows D..57 -> zbuf
                zbuf = xpool.tile([P, 2, 4, ZR, NT], FP8, name="zbuf", bufs=3)
                zbufs[img] = zbuf
                xpad = xpads[img]
                for cg in range(2):
                    A = xpad[:, cg, D:HP, 0:58].rearrange(
                        "p r (t s) -> p r t s", s=2
                    )
                    d0 = A[:, :, 0:NT, 0]
                    d1 = A[:, :, 0:NT, 1]
                    d2 = A[:, :, 1 : NT + 1, 0]
                    d3 = A[:, :, 1 : NT + 1, 1]
                    nc.vector.tensor_sub(zbuf[:, cg, 0], d0, d2)
                    nc.vector.tensor_add(zbuf[:, cg, 1], d1, d2)
                    nc.vector.tensor_sub(zbuf[:, cg, 2], d2, d1)
                    nc.vector.tensor_sub(zbuf[:, cg, 3], d1, d3)

            def emit_direct_group(img, h0, mt, eng):
                xpad = xpads[img]
                acc = pp.tile([P, CHUNK * W], F32, name="acc", bufs=ACCB)
                k = 0
                for kh in range(3):
                    for kw in range(3):
                        nc.tensor.matmul(
                            acc,
                            w_lhsT[:, :, mt, kh * 3 + kw, :],
                            xpad[:, :, h0 + kh : h0 + kh + CHUNK, kw : kw + W],
                            start=(k == 0),
                            stop=(k == 8),
                            perf_mode=mybir.MatmulPerfMode.DoubleRow,
                        )
                        k += 1
                ot = opool.tile([P, CHUNK, W], F32, name="ot", bufs=8)
                if eng == "dve":
                    nc.vector.tensor_scalar_mul(
                        out=ot,
                        in0=acc.rearrange("p (r c) -> p r c", c=W),
                        scalar1=alpha_sb[:, mt : mt + 1],
                    )
                else:
                    nc.scalar.mul(
                        ot,
                        acc.rearrange("p (r c) -> p r c", c=W),
                        alpha_sb[:, mt : mt + 1],
                    )
                nc.sync.dma_start(
                    out=y_ap[img, mt * P : (mt + 1) * P, h0 : h0 + CHUNK, :],
                    in_=ot,
                )

            def emit_wino_group(img, mt):
                # 16 output rows D..55 via F(2,3): m_j accumulate in psum
                # pairs; inverse on DVE; alpha on ACT; ~1.5x fewer PE cycles
                zbuf = zbufs[img]
                a01 = pp.tile([P, 2, 512], F32, name="waccp", bufs=WACCB)
                a23 = pp.tile([P, 2, 512], F32, name="waccp", bufs=WACCB)
                planes = [
                    (a01[:, 0, 0:448], 0),
                    (a01[:, 1, 0:448], 1),
                    (a23[:, 0, 0:448], 2),
                    (a23[:, 1, 0:448], 3),
                ]
                for accv, j in planes:
                    for kh in range(3):
                        if j == 0:
                            tap = kh * 3  # kw=0 direct tap
                        elif j == 3:
                            tap = kh * 3 + 2  # kw=2 direct tap
                        else:
                            tap = 9 + kh * 2 + (j - 1)
                        nc.tensor.matmul(
                            accv,
                            w_lhsT[:, :, mt, tap, :],
                            zbuf[:, :, j, kh : kh + 16, :],
                            start=(kh == 0),
                            stop=(kh == 2),
                            perf_mode=mybir.MatmulPerfMode.DoubleRow,
                        )
                m0 = a01[:, 0, 0:448]
                m1 = a01[:, 1, 0:448]
                m2 = a23[:, 0, 0:448]
                m3 = a23[:, 1, 0:448]
                # DVE may read only ONE operand from PSUM per op: stage m1
                # in SBUF via ACT first (also frees the a01 banks early)
                s1 = opool.tile([P, 16 * NT], F32, name="s1", bufs=2)
                su = opool.tile([P, 16 * NT], F32, name="su", bufs=2)
                sv = opool.tile([P, 16 * NT], F32, name="sv", bufs=2)
                otw = opool.tile([P, 16, W], F32, name="otw", bufs=2)
                OV = otw.rearrange("p r (t s) -> p r t s", s=2)

                def v3(ap):
                    return ap.rearrange("p (r t) -> p r t", t=NT)

                nc.scalar.copy(s1, m1)
                nc.vector.tensor_add(su, m0, s1)  # m0+m1
                nc.vector.tensor_add(OV[:, :, :, 0], v3(su), v3(m2))  # even
                nc.vector.tensor_sub(sv, s1, m2)  # m1-m2
                nc.vector.tensor_sub(OV[:, :, :, 1], v3(sv), v3(m3))  # odd
                otw2 = opool.tile([P, 16, W], F32, name="otw2", bufs=2)
                nc.scalar.mul(otw2, otw, alpha_sb[:, mt : mt + 1])
                nc.sync.dma_start(
                    out=y_ap[img, mt * P : (mt + 1) * P, D:H, :],
                    in_=otw2,
                )

            # ---------------- head ----------------
            emit_wdma(0, 0)
            emit_wdma(0, 1)
            emit_xpad(0)
            for q in range(4):
                for cg in range(2):
                    xq = emit_load_q(0, cg, q)
                    emit_sign(0, cg, q * 14, 14, xq)
            emit_wdma(1, 0)
            emit_wdma(1, 1)
            emit_xpad(1)
            x1srcs = [emit_load(1, cg) for cg in range(2)]

            emit_wsgn(0, 0)
            emit_wsgn(0, 1)
            emit_wtrans(0)
            if WINO:
                emit_wino_taps(0)

            # img0 mt0 direct; early copybacks on DVE (ACT is busy signing)
            for ci in range(ND):
                emit_direct_group(0, ci * CHUNK, 0, "dve" if ci < 2 else "act")

            # img1 signs (ACT, after img0's copybacks in queue order)
            for cg in range(2):
                emit_sign(1, cg, 0, H, x1srcs[cg])

            # mt1 weight prep, then img0 mt1
            emit_wsgn(1, 0)
            emit_wsgn(1, 1)
            emit_wtrans(1)
            if WINO:
                emit_wino_taps(1)
                emit_fwd(0)
                emit_fwd(1)
            # img0's wino groups spread between the mt1 tail groups so
            # their psum pair-ring drains before img1's first wino group
            for ci in range(3):
                emit_direct_group(0, ci * CHUNK, 1, "dve" if ci % 2 else "act")
            if WINO:
                emit_wino_group(0, 0)
            emit_direct_group(0, 3 * CHUNK, 1, "dve")
            emit_direct_group(0, 4 * CHUNK, 1, "act")
            if WINO:
                emit_wino_group(0, 1)

            # ---------------- steady state ----------------
            # per image: [wino mt0][direct mt0][wino mt1][direct mt1] with
            # fwd(img+1) emitted mid-image so it lands before img+1 starts
            for img in range(1, IMG_PER_CORE):
                if img + 1 < IMG_PER_CORE:
                    nxt = img + 1
                    emit_xpad(nxt)
                    nxt_srcs = [emit_load(nxt, cg) for cg in range(2)]
                else:
                    nxt = None
                if WINO:
                    emit_wino_group(img, 0)
                for ci in range(ND):
                    emit_direct_group(img, ci * CHUNK, 0, "dve" if ci % 2 else "act")
                if nxt is not None:
                    for cg in range(2):
                        emit_sign(nxt, cg, 0, H, nxt_srcs[cg])
                if WINO:
                    emit_wino_group(img, 1)
                    if nxt is not None:
                        emit_fwd(nxt)
                for ci in range(ND):
                    emit_direct_group(img, ci * CHUNK, 1, "act" if ci % 2 else "dve")
    nc.compile()
    return nc


def kernel(x, weight, alpha, trace=False):
    global last_result
    x = np.ascontiguousarray(np.asarray(x, dtype=np.float32))
    weight = np.ascontiguousarray(np.asarray(weight, dtype=np.float32))
    alpha = np.ascontiguousarray(np.asarray(alpha, dtype=np.float32))

    nc = build_conv_kernel()
    in_maps = [
        {
            "x": np.ascontiguousarray(x[i * IMG_PER_CORE : (i + 1) * IMG_PER_CORE]),
            "w": weight,
            "alpha": alpha,
        }
        for i in range(N_CORES)
    ]
    res = run_bass_kernel_spmd(nc, in_maps, list(range(N_CORES)), trace=trace)
    last_result = res
    out = np.concatenate([res.results[i]["y"] for i in range(N_CORES)], axis=0)
    return out.astype(np.float32, copy=False)


# revision 15
# speedup vs baseline: 1.0435x; 1.0435x over previous
"""XNOR-Net conv2d kernel for Trainium2.

Computes conv2d(sign(x), sign(W), stride=1, pad=1) * alpha for
x:(32,256,56,56) f32, W:(256,256,3,3) f32, alpha:(256,1,1) f32.

Strategy: data-parallel over batch (4 images per core x 8 cores).
Per core, implicit GEMM on the PE array in fp8 (DoubleRow, K=256).
sign(x) is +-1 in fp8 (exact); weights are carried as +-0.5 (one-pass
DVE compute (w>0)-0.5); the missing x2 is folded into alpha at
copyback, so results are bit-exact vs the reference.

Hybrid algorithm per image: rows [0, 40) direct (9 taps per 8-row
chunk, N=448 matmuls), rows [40, 56) via 1D Winograd F(2,3) along
width (12 taps per 16-row chunk, N=448), cutting PE cycles 1.5x for
that band. All Winograd values are exact in fp8: z in {-2..2},
transformed weights in {+-0.25,+-0.75} (built on DVE from the
transposed direct taps: j1=(t0+t1+t2)/2, j2=j1-t1), PSUM accumulates
multiples of 1/4 in fp32, so the result stays bit-exact.

Engine budget per image (target < PE ~22us): PE = 2x(5 direct + 1
wino group); DVE = forward transform + inverse adds + half the direct
copybacks; ACT = signs + other half + wino alpha + the m1 PSUM->SBUF
stage (DVE tensor_tensor may read only one PSUM operand). DMA: few
large transfers (12.5KB/partition image lines, 4.6KB weight lines).
Emission order per engine matches expected execution order (queues
are in-order; a blocked head stalls the queue).
"""

import sys

sys.path.insert(0, "/opt/trn_rl_repo")

import numpy as np

import concourse.bass as bass
import concourse.mybir as mybir
from concourse import bacc
from concourse.bass_utils import run_bass_kernel_spmd
from concourse.masks import make_identity
from concourse.tile import TileContext

P = 128
N_CORES = 8
N_IMG = 32
IMG_PER_CORE = N_IMG // N_CORES
C = 256
H = W = 56
HP = 58  # padded rows (0..57)
WS = 64  # row stride of padded buffer (cols 0..57 used, 58+ never read)
CHUNK = 8  # direct-path output rows per matmul tile -> N = 8*56 = 448
NT = W // 2  # winograd tiles per row (28)
FP8 = mybir.dt.float8e4
F32 = mybir.dt.float32
BF16 = mybir.dt.bfloat16

WINO = 16  # rows per image via Winograd F(2,3); 0 or 16
D = H - WINO  # direct rows 0..D-1
ND = D // CHUNK  # direct chunks
ZR = WINO + 2  # z rows needed (output rows D..55 + kh halo)
NTAP = 15 if WINO else 9  # 9 direct taps (+6 winograd j1/j2 taps)
ACCB = 4 if WINO else 8  # direct-acc PSUM ring (also transpose targets)
WACCB = 2  # winograd acc-pair ring (2 banks each)

last_result = None  # stash of BassKernelResults for test harnesses


def build_conv_kernel():
    nc = bacc.Bacc()
    x_in = nc.declare_dram_parameter(
        "x", [IMG_PER_CORE, C, H, W], F32, isOutput=False
    )
    w_in = nc.declare_dram_parameter("w", [C, C, 3, 3], F32, isOutput=False)
    a_in = nc.declare_dram_parameter("alpha", [C, 1, 1], F32, isOutput=False)
    y_out = nc.declare_dram_parameter(
        "y", [IMG_PER_CORE, C, H, W], F32, isOutput=True
    )
    x_ap, w_ap, a_ap, y_ap = x_in[:], w_in[:], a_in[:], y_out[:]

    with TileContext(nc) as tc:
        with (
            tc.tile_pool(name="wpool", bufs=1) as wpool,
            tc.tile_pool(name="xpool", bufs=3) as xpool,
            tc.tile_pool(name="opool", bufs=6) as opool,
            tc.tile_pool(name="pp", bufs=1, space="PSUM") as pp,
        ):
            # warm up the ACT function table while the first DMAs run
            warm = wpool.tile([P, 1], F32, name="warm")
            nc.vector.memset(warm, 0.0)
            nc.scalar.sign(warm, warm)

            ident = wpool.tile([P, P], BF16, name="ident")
            make_identity(nc, ident)
            alpha_sb = wpool.tile([P, 2], F32, name="alpha_sb")
            nc.sync.dma_start(
                out=alpha_sb, in_=a_ap.flatten().rearrange("(mt co) -> co mt", co=P)
            )
            # weights carry +-0.5; restore the factor 2 here
            nc.vector.tensor_scalar(
                out=alpha_sb,
                in0=alpha_sb,
                scalar1=2.0,
                scalar2=None,
                op0=mybir.AluOpType.mult,
            )

            # [ci_lo, cg, mt, tap, co]; taps 0-8 = (kh*3+kw), taps 9-14 =
            # winograd (9+kh*2 = j1, 10+kh*2 = j2)
            w_lhsT = wpool.tile([P, 2, 2, NTAP, P], FP8, name="w_lhsT")

            wsrcs = {}
            wsgns = {}

            def emit_wdma(mt, cg):
                # [co, ci*9] with 4.6KB contiguous lines
                wsrc = wpool.tile([P, P * 9], F32, name="wsrc", bufs=4)
                nc.sync.dma_start(
                    out=wsrc,
                    in_=w_ap[
                        mt * P : (mt + 1) * P, cg * P : (cg + 1) * P
                    ].rearrange("co ci kh kw -> co (ci kh kw)"),
                )
                wsrcs[(mt, cg)] = wsrc

            def emit_wsgn(mt, cg):
                # one-pass half-sign on DVE: (w > 0) - 0.5 -> +-0.5
                wsgn = wpool.tile([P, P * 9], BF16, name="wsgn", bufs=4)
                nc.vector.tensor_scalar(
                    out=wsgn,
                    in0=wsrcs[(mt, cg)],
                    scalar1=0.0,
                    scalar2=0.5,
                    op0=mybir.AluOpType.is_gt,
                    op1=mybir.AluOpType.subtract,
                )
                wsgns[(mt, cg)] = wsgn

            def emit_wtrans(mt):
                # tap-major, cg-interleaved so tap k (both cg planes) is
                # ready early; transpose targets alias direct-acc PSUM
                # slots (bf16 view); copies to fp8 all on DVE
                for tap in range(9):
                    for cg in range(2):
                        wsgn_v = wsgns[(mt, cg)].rearrange(
                            "p (ci t) -> p ci t", t=9
                        )
                        accb = pp.tile([P, CHUNK * W], F32, name="acc", bufs=ACCB)
                        tpv = accb[:, 0:64].bitcast(BF16)
                        nc.tensor.transpose(tpv, wsgn_v[:, :, tap], ident)
                        nc.vector.tensor_copy(
                            out=w_lhsT[:, cg, mt, tap, :], in_=tpv
                        )

            def emit_wino_taps(mt):
                # winograd taps from transposed direct taps, on DVE:
                # j1 = (t0+t1+t2)/2, j2 = j1-t1 (+-0.25/+-0.75; exact)
                for cg in range(2):
                    dir_v = w_lhsT[:, cg, mt, 0:9, :].rearrange(
                        "p (kh kw) co -> p kh kw co", kw=3
                    )
                    win_v = w_lhsT[:, cg, mt, 9:15, :].rearrange(
                        "p (kh j) co -> p kh j co", j=2
                    )
                    wu = wpool.tile([P, 3, P], FP8, name="wu", bufs=2)
                    wv = wpool.tile([P, 3, P], FP8, name="wv", bufs=2)
                    nc.vector.tensor_add(wu, dir_v[:, :, 0, :], dir_v[:, :, 2, :])
                    nc.vector.tensor_add(wv, wu, dir_v[:, :, 1, :])
                    nc.vector.tensor_scalar_mul(
                        out=win_v[:, :, 0, :], in0=wv, scalar1=0.5
                    )
                    nc.vector.tensor_sub(
                        win_v[:, :, 1, :], win_v[:, :, 0, :], dir_v[:, :, 1, :]
                    )

            xpads = {}
            zbufs = {}

            def emit_xpad(img):
                xpad = xpool.tile([P, 2, HP, WS], FP8, name="xpad", bufs=3)
                xpads[img] = xpad
                nc.vector.memset(xpad[:, :, 0, 0:58], 0.0)
                nc.vector.memset(xpad[:, :, HP - 1, 0:58], 0.0)
                nc.vector.memset(xpad[:, :, 1 : HP - 1, 0], 0.0)
                nc.vector.memset(xpad[:, :, 1 : HP - 1, 57], 0.0)

            def emit_load(img, cg):
                xsrc = xpool.tile([P, H * W], F32, name="xsrc", bufs=4)
                nc.sync.dma_start(
                    out=xsrc,
                    in_=x_ap[img, cg * P : (cg + 1) * P].rearrange(
                        "c h w -> c (h w)"
                    ),
                )
                return xsrc

            def emit_load_q(img, cg, q):
                xq = xpool.tile([P, 14 * W], F32, name="xsrcq", bufs=4)
                nc.sync.dma_start(
                    out=xq,
                    in_=x_ap[
                        img, cg * P : (cg + 1) * P, q * 14 : (q + 1) * 14
                    ].rearrange("c h w -> c (h w)"),
                )
                return xq

            def emit_sign(img, cg, r0, nr, src):
                nc.scalar.sign(
                    xpads[img][:, cg, r0 + 1 : r0 + 1 + nr, 1 : W + 1],
                    src.rearrange("p (h w) -> p h w", w=W),
                )

            def emit_fwd(img):
                # forward winograd transform of padded rows D..57 -> zbuf
                zbuf = xpool.tile([P, 2, 4, ZR, NT], FP8, name="zbuf", bufs=3)
                zbufs[img] = zbuf
                xpad = xpads[img]
                for cg in range(2):
                    A = xpad[:, cg, D:HP, 0:58].rearrange(
                        "p r (t s) -> p r t s", s=2
                    )
                    d0 = A[:, :, 0:NT, 0]
                    d1 = A[:, :, 0:NT, 1]
                    d2 = A[:, :, 1 : NT + 1, 0]
                    d3 = A[:, :, 1 : NT + 1, 1]
                    nc.vector.tensor_sub(zbuf[:, cg, 0], d0, d2)
                    nc.vector.tensor_add(zbuf[:, cg, 1], d1, d2)
                    nc.vector.tensor_sub(zbuf[:, cg, 2], d2, d1)
                    nc.vector.tensor_sub(zbuf[:, cg, 3], d1, d3)

            def emit_direct_group(img, h0, mt, eng):
                xpad = xpads[img]
                acc = pp.tile([P, CHUNK * W], F32, name="acc", bufs=ACCB)
                k = 0
                for kh in range(3):
                    for kw in range(3):
                        nc.tensor.matmul(
                            acc,
                            w_lhsT[:, :, mt, kh * 3 + kw, :],
                            xpad[:, :, h0 + kh : h0 + kh + CHUNK, kw : kw + W],
                            start=(k == 0),
                            stop=(k == 8),
                            perf_mode=mybir.MatmulPerfMode.DoubleRow,
                        )
                        k += 1
                ot = opool.tile([P, CHUNK, W], F32, name="ot", bufs=8)
                if eng == "dve":
                    nc.vector.tensor_scalar_mul(
                        out=ot,
                        in0=acc.rearrange("p (r c) -> p r c", c=W),
                        scalar1=alpha_sb[:, mt : mt + 1],
                    )
                else:
                    nc.scalar.mul(
                        ot,
                        acc.rearrange("p (r c) -> p r c", c=W),
                        alpha_sb[:, mt : mt + 1],
                    )
                nc.sync.dma_start(
                    out=y_ap[img, mt * P : (mt + 1) * P, h0 : h0 + CHUNK, :],
                    in_=ot,
                )

            def emit_wino_group(img, mt):
                # 16 output rows D..55 via F(2,3): m_j accumulate in psum
                # pairs; inverse on DVE; alpha on ACT; ~1.5x fewer PE cycles
                zbuf = zbufs[img]
                a01 = pp.tile([P, 2, 512], F32, name="waccp", bufs=WACCB)
                a23 = pp.tile([P, 2, 512], F32, name="waccp", bufs=WACCB)
                planes = [
                    (a01[:, 0, 0:448], 0),
                    (a01[:, 1, 0:448], 1),
                    (a23[:, 0, 0:448], 2),
                    (a23[:, 1, 0:448], 3),
                ]
                for accv, j in planes:
                    for kh in range(3):
                        if j == 0:
                            tap = kh * 3  # kw=0 direct tap
                        elif j == 3:
                            tap = kh * 3 + 2  # kw=2 direct tap
                        else:
                            tap = 9 + kh * 2 + (j - 1)
                        nc.tensor.matmul(
                            accv,
                            w_lhsT[:, :, mt, tap, :],
                            zbuf[:, :, j, kh : kh + 16, :],
                            start=(kh == 0),
                            stop=(kh == 2),
                            perf_mode=mybir.MatmulPerfMode.DoubleRow,
                        )
                m0 = a01[:, 0, 0:448]
                m1 = a01[:, 1, 0:448]
                m2 = a23[:, 0, 0:448]
                m3 = a23[:, 1, 0:448]
                # DVE may read only ONE operand from PSUM per op: stage m1
                # in SBUF via ACT first (also frees the a01 banks early)
                s1 = opool.tile([P, 16 * NT], F32, name="s1", bufs=2)
                su = opool.tile([P, 16 * NT], F32, name="su", bufs=2)
                sv = opool.tile([P, 16 * NT], F32, name="sv", bufs=2)
                otw = opool.tile([P, 16, W], F32, name="otw", bufs=2)
                OV = otw.rearrange("p r (t s) -> p r t s", s=2)

                def v3(ap):
                    return ap.rearrange("p (r t) -> p r t", t=NT)

                nc.scalar.copy(s1, m1)
                nc.vector.tensor_add(su, m0, s1)  # m0+m1
                nc.vector.tensor_add(OV[:, :, :, 0], v3(su), v3(m2))  # even
                nc.vector.tensor_sub(sv, s1, m2)  # m1-m2
                nc.vector.tensor_sub(OV[:, :, :, 1], v3(sv), v3(m3))  # odd
                otw2 = opool.tile([P, 16, W], F32, name="otw2", bufs=2)
                nc.scalar.mul(otw2, otw, alpha_sb[:, mt : mt + 1])
                nc.sync.dma_start(
                    out=y_ap[img, mt * P : (mt + 1) * P, D:H, :],
                    in_=otw2,
                )

            # ---------------- head ----------------
            emit_wdma(0, 0)
            emit_wdma(0, 1)
            emit_xpad(0)
            for q in range(4):
                for cg in range(2):
                    xq = emit_load_q(0, cg, q)
                    emit_sign(0, cg, q * 14, 14, xq)
            emit_wdma(1, 0)
            emit_wdma(1, 1)
            emit_xpad(1)
            x1srcs = [emit_load(1, cg) for cg in range(2)]

            emit_wsgn(0, 0)
            emit_wsgn(0, 1)
            emit_wtrans(0)
            if WINO:
                emit_wino_taps(0)

            # img0 mt0 direct; early copybacks on DVE (ACT is busy signing)
            for ci in range(ND):
                emit_direct_group(0, ci * CHUNK, 0, "dve" if ci < 2 else "act")

            # img1 signs (ACT, after img0's copybacks in queue order)
            for cg in range(2):
                emit_sign(1, cg, 0, H, x1srcs[cg])

            # mt1 weight prep, then img0 mt1
            emit_wsgn(1, 0)
            emit_wsgn(1, 1)
            emit_wtrans(1)
            if WINO:
                emit_wino_taps(1)
                emit_fwd(0)
                emit_fwd(1)
            for ci in range(ND):
                emit_direct_group(0, ci * CHUNK, 1, "dve" if ci % 2 else "act")
            if WINO:
                emit_wino_group(0, 0)
                emit_wino_group(0, 1)

            # ---------------- steady state ----------------
            # per image: [wino mt0][direct mt0][wino mt1][direct mt1] with
            # fwd(img+1) emitted mid-image so it lands before img+1 starts
            for img in range(1, IMG_PER_CORE):
                if img + 1 < IMG_PER_CORE:
                    nxt = img + 1
                    emit_xpad(nxt)
                    nxt_srcs = [emit_load(nxt, cg) for cg in range(2)]
                else:
                    nxt = None
                if WINO:
                    emit_wino_group(img, 0)
                for ci in range(ND):
                    emit_direct_group(img, ci * CHUNK, 0, "dve" if ci % 2 else "act")
                if nxt is not None:
                    for cg in range(2):
                        emit_sign(nxt, cg, 0, H, nxt_srcs[cg])
                if WINO:
                    emit_wino_group(img, 1)
                    if nxt is not None:
                        emit_fwd(nxt)
                for ci in range(ND):
                    emit_direct_group(img, ci * CHUNK, 1, "act" if ci % 2 else "dve")
    nc.compile()
    return nc


def kernel(x, weight, alpha, trace=False):
    global last_result
    x = np.ascontiguousarray(np.asarray(x, dtype=np.float32))
    weight = np.ascontiguousarray(np.asarray(weight, dtype=np.float32))
    alpha = np.ascontiguousarray(np.asarray(alpha, dtype=np.float32))

    nc = build_conv_kernel()
    in_maps = [
        {
            "x": np.ascontiguousarray(x[i * IMG_PER_CORE : (i + 1) * IMG_PER_CORE]),
            "w": weight,
            "alpha": alpha,
        }
        for i in range(N_CORES)
    ]
    res = run_bass_kernel_spmd(nc, in_maps, list(range(N_CORES)), trace=trace)
    last_result = res
    out = np.concatenate([res.results[i]["y"] for i in range(N_CORES)], axis=0)
    return out.astype(np.float32, copy=False)
